# revision 13
# baseline (speedup 1.0000x reference)
"""Trainium2 Bass kernel for nn_ActivationAndBiophysModel.

2-layer GRU (H=512) + FC + antagonist-muscle biophysics, T=512 steps, B=64.

Strategy notes (why this shape):
- The recurrence is strictly sequential over T=512; the per-step compute is
  small-matrix. On-chip collectives have a ~4.6us floor per call, so any
  cross-core exchange per timestep (1024 of them) costs more than the whole
  computation. Every core therefore runs the full model replicated; core 0's
  output is returned.
- Matmuls use "Form A": stationary operand = h^T (cheap to load, [K<=128, 64]),
  moving operand = weight panels streamed at 1 col/cycle. Cost is then just
  (weight elements / 128) cycles per step, independent of batch size <= 128.
- h^T is regenerated each step from the batch-major h via PE transpose.
- All biases are folded into ones-row matmuls (no extra vector ops).
- The biophysics integrator is algebraically collapsed into 9 fused DVE ops
  using scalar_tensor_tensor.
"""

import sys

for p in ("/opt/trn_rl_repo", "/opt/pypackages"):
    if p not in sys.path:
        sys.path.insert(0, p)

import numpy as np  # noqa: E402

B, T, IN, H, J = 64, 512, 16, 512, 8
HG = 3 * H  # 1536 gate rows

# muscle / joint constants (bilinearInit in the model)
K0, K1, L0m, L1m, Mm = 100.0, 2000.0, 0.06, 0.006, 0.05
Ij, Kj, Bj, DT = 0.004, 5.0, 0.3, 1.0 / 60.0

# collapsed integrator coefficients:
# s = a1+a0, d = a1-a0, p = s*d
# om' = ALPHA*om + BETA*d + GAMMA*p + DELTA*th + EPS*(s*th);  th' = th + DT*om'
_c = DT / Ij
ALPHA = 1.0 - _c * Bj
BETA = _c * Mm * (K0 * L1m + K1 * L0m)
GAMMA = _c * Mm * K1 * L1m
DELTA = _c * (-(2.0 * Mm * Mm * K0) - Kj)
EPS = _c * (-(Mm * Mm * K1))


def _build(nc, bass, tile, mybir, T_run, mv_dt, unroll_dma=8):
    """Emit the full unrolled program into nc."""
    f32 = mybir.dt.float32
    AF = mybir.ActivationFunctionType
    Alu = mybir.AluOpType

    def mm(out, lhsT, rhs, **kw):
        nc.tensor.matmul(out, lhsT.bitcast(mv_dt), rhs.bitcast(mv_dt), **kw)

    # ---- DRAM parameters -------------------------------------------------
    xT_d = nc.declare_dram_parameter("xT", [IN + 1, T_run, B], mv_dt, isOutput=False)
    w0x_d = nc.declare_dram_parameter("w0x", [IN + 1, HG], mv_dt, isOutput=False)
    w0h_d = nc.declare_dram_parameter("w0h", [4, 128, HG], mv_dt, isOutput=False)
    w1i_d = nc.declare_dram_parameter("w1i", [4, 128, HG], mv_dt, isOutput=False)
    w1h_d = nc.declare_dram_parameter("w1h", [4, 128, HG], mv_dt, isOutput=False)
    wfc_d = nc.declare_dram_parameter("wfc", [4, 128, 2 * J], mv_dt, isOutput=False)
    brows_d = nc.declare_dram_parameter("brows", [1, 2576 + B], mv_dt, isOutput=False)
    ident_d = nc.declare_dram_parameter("ident", [B, B], f32, isOutput=False)
    hb0_d = nc.declare_dram_parameter("hb0", [2, B, H], f32, isOutput=False)
    hT0_d = nc.declare_dram_parameter("hT0", [2, 4, 128, B], mv_dt, isOutput=False)
    th0_d = nc.declare_dram_parameter("th0", [B, J], f32, isOutput=False)
    om0_d = nc.declare_dram_parameter("om0", [B, J], f32, isOutput=False)
    out_d = nc.declare_dram_parameter("out", [B, T_run * J], f32, isOutput=True)

    # brows layout offsets
    OB1RZ, OB1IN, OB1HN, OB0HN, OBFC = 0, 1024, 1536, 2048, 2560

    with tile.TileContext(nc) as tc:
        with (
            tc.tile_pool(name="wpool", bufs=1) as wp,
            tc.tile_pool(name="xpool", bufs=8) as xp,
            tc.tile_pool(name="state", bufs=2) as sp,
            tc.tile_pool(name="gates", bufs=2) as gp,
            tc.tile_pool(name="bp", bufs=2) as bp,
            tc.tile_pool(name="prz", bufs=2, space="PSUM") as prz,
            tc.tile_pool(name="psm", bufs=4, space="PSUM") as psm,
        ):
            # ---- load constants/weights once -----------------------------
            w0x = wp.tile([IN + 1, HG], mv_dt)
            nc.sync.dma_start(w0x[:], w0x_d[:])
            w0h = wp.tile([128, 4, HG], mv_dt)
            w1i = wp.tile([128, 4, HG], mv_dt)
            w1h = wp.tile([128, 4, HG], mv_dt)
            wfc = wp.tile([128, 4, 2 * J], mv_dt)
            for c in range(4):
                nc.sync.dma_start(w0h[:, c, :], w0h_d[c])
                nc.sync.dma_start(w1i[:, c, :], w1i_d[c])
                nc.sync.dma_start(w1h[:, c, :], w1h_d[c])
                nc.sync.dma_start(wfc[:, c, :], wfc_d[c])
            brows = wp.tile([1, 2576 + B], mv_dt)
            nc.sync.dma_start(brows[:], brows_d[:])
            ident = wp.tile([B, B], f32)
            nc.sync.dma_start(ident[:], ident_d[:])
            out_sb = wp.tile([B, T_run * J], f32)
            ones = brows[:, 2576 : 2576 + B]

            # ---- initial state -------------------------------------------
            h0b = sp.tile([B, H], f32, tag="h0b")
            h1b = sp.tile([B, H], f32, tag="h1b")
            nc.sync.dma_start(h0b[:], hb0_d[0])
            nc.sync.dma_start(h1b[:], hb0_d[1])
            h0T = sp.tile([128, 4 * B], mv_dt, tag="h0T")
            h1T = sp.tile([128, 4 * B], mv_dt, tag="h1T")
            for c in range(4):
                nc.sync.dma_start(h0T[:, c * B : (c + 1) * B], hT0_d[0, c])
                nc.sync.dma_start(h1T[:, c * B : (c + 1) * B], hT0_d[1, c])
            th_init = sp.tile([B, J], f32, tag="th")
            nc.sync.dma_start(th_init[:], th0_d[:])
            om = sp.tile([B, J], f32, tag="om")
            nc.sync.dma_start(om[:], om0_d[:])
            th_ap = th_init[:]

            xts = [None] * (T_run + 2)

            def dma_x(t):
                if t < T_run and xts[t] is None:
                    xt = xp.tile([IN + 1, B], mv_dt, tag="xt")
                    nc.sync.dma_start(xt[:], xT_d[:, t, :])
                    xts[t] = xt

            def l0_rz_mms(rz, h0T_ap, xt):
                sl = [rz[:, 0:512], rz[:, 512:1024]]
                for c in range(4):
                    st = h0T_ap[:, c * B : (c + 1) * B]
                    for ns in range(2):
                        mm(sl[ns], st, w0h[:, c, ns * 512 : ns * 512 + 512],
                           start=(c == 0), stop=False)
                for ns in range(2):
                    mm(sl[ns], xt[:], w0x[:, ns * 512 : ns * 512 + 512],
                       start=False, stop=True)

            def l0_ni_mms(hn, inn, h0T_ap, xt):
                for c in range(4):
                    mm(hn[:], h0T_ap[:, c * B : (c + 1) * B], w0h[:, c, 1024:1536],
                       start=(c == 0), stop=False)
                mm(inn[:], xt[:], w0x[:, 1024:1536], start=True, stop=True)
                mm(hn[:], ones, brows[:, OB0HN : OB0HN + 512],
                   start=False, stop=True)

            def l1_hh_mms(rz, hn, h1T_ap):
                sl = [rz[:, 0:512], rz[:, 512:1024]]
                for c in range(4):
                    st = h1T_ap[:, c * B : (c + 1) * B]
                    for ns in range(2):
                        mm(sl[ns], st, w1h[:, c, ns * 512 : ns * 512 + 512],
                           start=(c == 0), stop=False)
                    mm(hn[:], st, w1h[:, c, 1024:1536], start=(c == 0), stop=False)

            def l1_ih_mms(rz, hn, inn, h0T_ap):
                sl = [rz[:, 0:512], rz[:, 512:1024]]
                for c in range(4):
                    st = h0T_ap[:, c * B : (c + 1) * B]
                    for ns in range(2):
                        mm(sl[ns], st, w1i[:, c, ns * 512 : ns * 512 + 512],
                           start=False, stop=False)
                    mm(inn[:], st, w1i[:, c, 1024:1536], start=(c == 0), stop=False)
                for ns in range(2):
                    mm(sl[ns], ones, brows[:, OB1RZ + ns * 512 : OB1RZ + ns * 512 + 512],
                       start=False, stop=True)
                mm(hn[:], ones, brows[:, OB1HN : OB1HN + 512], start=False, stop=True)
                mm(inn[:], ones, brows[:, OB1IN : OB1IN + 512], start=False, stop=True)

            def gru_vec(rz, hn, inn, lo, hb_prev, hb_tag):
                r_ = gp.tile([B, H], f32, tag=f"r{lo}")
                nc.scalar.activation(r_[:], rz[:, 0:512], AF.Sigmoid)
                z_ = gp.tile([B, H], f32, tag=f"z{lo}")
                nc.scalar.activation(z_[:], rz[:, 512:1024], AF.Sigmoid)
                zb = gp.tile([B, H], f32, tag=f"zb{lo}")
                nc.scalar.activation(zb[:], rz[:, 512:1024], AF.Sigmoid, scale=-1.0)
                q2 = gp.tile([B, H], f32, tag=f"q2{lo}")
                nc.vector.tensor_mul(q2[:], z_[:], hb_prev[:])
                t_ = gp.tile([B, H], f32, tag=f"t{lo}")
                nc.vector.tensor_mul(t_[:], r_[:], hn[:])
                u_ = gp.tile([B, H], f32, tag=f"u{lo}")
                nc.vector.tensor_add(u_[:], t_[:], inn[:])
                n_ = gp.tile([B, H], f32, tag=f"n{lo}")
                nc.scalar.activation(n_[:], u_[:], AF.Tanh)
                q1 = gp.tile([B, H], f32, tag=f"q1{lo}")
                nc.vector.tensor_mul(q1[:], zb[:], n_[:])
                hb = sp.tile([B, H], f32, tag=hb_tag)
                nc.vector.tensor_add(hb[:], q1[:], q2[:])
                return hb

            def transpose_h(hb, hT_tag):
                ph = psm.tile([128, 4 * B], f32, tag="sm")
                for c in range(4):
                    nc.tensor.transpose(ph[:, c * B : (c + 1) * B],
                                        hb[:, c * 128 : (c + 1) * 128], ident[:])
                hT = sp.tile([128, 4 * B], mv_dt, tag=hT_tag)
                nc.scalar.activation(hT[:], ph[:], AF.Copy)
                return hT

            # ---- prologue: step 0 gate matmuls ---------------------------
            dma_x(0)
            dma_x(1)
            rz0 = prz.tile([B, 1024], f32, tag="rz")
            l0_rz_mms(rz0, h0T, xts[0])
            hn0 = psm.tile([B, 512], f32, tag="sm")
            inn0 = psm.tile([B, 512], f32, tag="sm")
            l0_ni_mms(hn0, inn0, h0T, xts[0])
            rz1 = prz.tile([B, 1024], f32, tag="rz")
            hn1 = psm.tile([B, 512], f32, tag="sm")
            l1_hh_mms(rz1, hn1, h1T)

            # ---- time loop (fully unrolled, 2-deep software pipeline) ----
            for t in range(T_run):
                dma_x(t + 2)
                last = t + 1 >= T_run

                # vec layer0(t) + transpose
                h0b = gru_vec(rz0, hn0, inn0, 0, h0b, "h0b")
                h0T = transpose_h(h0b, "h0T")

                # phase A: layer0(t+1) rz matmuls; layer1(t) ih matmuls
                if not last:
                    rz0_n = prz.tile([B, 1024], f32, tag="rz")
                    l0_rz_mms(rz0_n, h0T, xts[t + 1])
                inn1 = psm.tile([B, 512], f32, tag="sm")
                l1_ih_mms(rz1, hn1, inn1, h0T)

                # vec layer1(t) + transpose
                h1b = gru_vec(rz1, hn1, inn1, 1, h1b, "h1b")
                h1T = transpose_h(h1b, "h1T")

                # phase B: fc(t); layer0(t+1) hn/inn; layer1(t+1) hh
                ps_fc = psm.tile([B, 2 * J], f32, tag="sm")
                fc = ps_fc[:, :]
                for c in range(4):
                    mm(fc, h1T[:, c * B : (c + 1) * B], wfc[:, c, :],
                       start=(c == 0), stop=False)
                mm(fc, ones, brows[:, OBFC : OBFC + 2 * J], start=False, stop=True)
                if not last:
                    hn0 = psm.tile([B, 512], f32, tag="sm")
                    inn0 = psm.tile([B, 512], f32, tag="sm")
                    l0_ni_mms(hn0, inn0, h0T, xts[t + 1])
                    rz1_n = prz.tile([B, 1024], f32, tag="rz")
                    hn1 = psm.tile([B, 512], f32, tag="sm")
                    l1_hh_mms(rz1_n, hn1, h1T)
                    rz0, rz1 = rz0_n, rz1_n

                # fc sigmoid + biophysics
                a_s = bp.tile([B, J, 2], f32, tag="as")
                nc.scalar.activation(a_s[:], fc, AF.Sigmoid)
                a0 = a_s[:, :, 0]
                a1 = a_s[:, :, 1]
                s_ = bp.tile([B, J], f32, tag="s")
                nc.gpsimd.tensor_add(s_[:], a1, a0)
                dd = bp.tile([B, J], f32, tag="dd")
                nc.gpsimd.tensor_sub(dd[:], a1, a0)
                p_ = bp.tile([B, J], f32, tag="p")
                nc.gpsimd.tensor_mul(p_[:], s_[:], dd[:])
                w_ = bp.tile([B, J], f32, tag="w")
                nc.vector.scalar_tensor_tensor(
                    w_[:], dd[:], BETA / GAMMA, p_[:], Alu.mult, Alu.add)
                v_ = bp.tile([B, J], f32, tag="v")
                nc.vector.tensor_scalar(v_[:], s_[:], EPS, DELTA, Alu.mult, Alu.add)
                u2 = bp.tile([B, J], f32, tag="u2")
                nc.vector.tensor_mul(u2[:], v_[:], th_ap)
                q_ = bp.tile([B, J], f32, tag="q")
                nc.vector.scalar_tensor_tensor(
                    q_[:], w_[:], GAMMA, u2[:], Alu.mult, Alu.add)
                om_new = sp.tile([B, J], f32, tag="om")
                nc.vector.scalar_tensor_tensor(
                    om_new[:], om[:], ALPHA, q_[:], Alu.mult, Alu.add)
                om = om_new
                th_new = out_sb[:, t * J : (t + 1) * J]
                nc.vector.scalar_tensor_tensor(
                    th_new, om[:], DT, th_ap, Alu.mult, Alu.add)
                th_ap = th_new

            nc.sync.dma_start(out_d[:], out_sb[:])
    return nc


_NC_CACHE = {}


def _get_nc(T_run):
    key = T_run
    if key in _NC_CACHE:
        return _NC_CACHE[key]
    from concourse import bass, bacc, tile

    mybir = bass.mybir
    mv_dt = mybir.dt.float32r  # fp32 data, 1 cycle/row streaming for N>=256
    nc = bacc.Bacc(None, target_bir_lowering=False)
    _build(nc, bass, tile, mybir, T_run, mv_dt)
    nc.compile()
    _NC_CACHE[key] = nc
    return nc


def _prep_inputs(x, W_ih0, W_hh0, b_ih0, b_hh0, W_ih1, W_hh1, b_ih1, b_hh1,
                 fc_W, fc_b, h0, theta0, omega0):
    T_run = x.shape[1]
    f = np.float32
    xT = np.concatenate(
        [np.ascontiguousarray(x.transpose(2, 1, 0)),
         np.ones((1, T_run, B), f)], axis=0).astype(f)  # [17, T, 64]
    b0rz = (b_ih0 + b_hh0)[:1024]
    w0x = np.concatenate(
        [W_ih0.T, np.concatenate([b0rz, b_ih0[1024:]])[None, :]], axis=0
    ).astype(f)  # [17, 1536]
    w0h = np.ascontiguousarray(W_hh0.T.reshape(4, 128, HG)).astype(f)
    w1i = np.ascontiguousarray(W_ih1.T.reshape(4, 128, HG)).astype(f)
    w1h = np.ascontiguousarray(W_hh1.T.reshape(4, 128, HG)).astype(f)
    wfc = np.ascontiguousarray(fc_W.T.reshape(4, 128, 2 * J)).astype(f)
    brows = np.zeros((1, 2576 + B), f)
    brows[0, 2576:] = 1.0
    brows[0, 0:1024] = (b_ih1 + b_hh1)[:1024]
    brows[0, 1024:1536] = b_ih1[1024:]
    brows[0, 1536:2048] = b_hh1[1024:]
    brows[0, 2048:2560] = b_hh0[1024:]
    brows[0, 2560:2576] = fc_b
    hT0 = np.ascontiguousarray(
        np.stack([h0[0].T.reshape(4, 128, B), h0[1].T.reshape(4, 128, B)])
    ).astype(f)
    return {
        "xT": xT, "w0x": w0x, "w0h": w0h, "w1i": w1i, "w1h": w1h,
        "wfc": wfc, "brows": brows, "ident": np.eye(B, dtype=f),
        "hb0": h0.astype(f), "hT0": hT0,
        "th0": theta0.astype(f), "om0": omega0.astype(f),
    }


def _install_loud_hook():
    """Surface compile-hook exceptions (XLA otherwise swallows them)."""
    import traceback

    from concourse import bass2jax

    if getattr(bass2jax, "_loud_hook_installed", False):
        return
    orig = bass2jax.neuronx_cc_hook

    def loud(*a, **k):
        try:
            return orig(*a, **k)
        except BaseException:
            traceback.print_exc()
            raise

    bass2jax.neuronx_cc_hook = loud
    bass2jax._loud_hook_installed = True

    # Enable walrus's LDWEIGHTS-dedup pass (concourse hardcodes it off).
    # Our matmuls are chunk-major so consecutive MMs share the stationary
    # operand; the dedup removes ~half the weight-load traffic.
    import os

    if os.environ.get("KERNEL_LDW_OPT", "1") == "1":
        from concourse import bass_utils as _bu

        if not getattr(_bu, "_ldw_patch", False):
            _orig_rc = _bu.run_command

            def _rc(cmd, **kw):
                cmd = [c.replace("--enable-ldw-opt=false", "--enable-ldw-opt=true")
                       if isinstance(c, str) else c for c in cmd]
                return _orig_rc(cmd, **kw)

            _bu.run_command = _rc
            _bu._ldw_patch = True


def run(inputs, **spmd_kwargs):
    from concourse.bass_utils import run_bass_kernel_spmd

    _install_loud_hook()

    inputs = {k: np.asarray(v) for k, v in inputs.items()}
    T_run = inputs["x"].shape[1]
    nc = _get_nc(T_run)
    in_map = _prep_inputs(**inputs)
    res = run_bass_kernel_spmd(nc, [in_map] * 8, core_ids=list(range(8)),
                               **spmd_kwargs)
    out = res.results[0]["out"].reshape(B, T_run, J).astype(np.float32)
    return out, res


def kernel(**inputs):
    return run(inputs)[0]


if __name__ == "__main__":
    rs = np.random.RandomState(0)
    demo = {
        "x": rs.randn(B, 8, IN).astype(np.float32),
        "W_ih0": 0.04 * rs.randn(HG, IN).astype(np.float32),
        "W_hh0": 0.04 * rs.randn(HG, H).astype(np.float32),
        "b_ih0": 0.04 * rs.randn(HG).astype(np.float32),
        "b_hh0": 0.04 * rs.randn(HG).astype(np.float32),
        "W_ih1": 0.04 * rs.randn(HG, H).astype(np.float32),
        "W_hh1": 0.04 * rs.randn(HG, H).astype(np.float32),
        "b_ih1": 0.04 * rs.randn(HG).astype(np.float32),
        "b_hh1": 0.04 * rs.randn(HG).astype(np.float32),
        "fc_W": 0.04 * rs.randn(2 * J, H).astype(np.float32),
        "fc_b": 0.04 * rs.randn(2 * J).astype(np.float32),
        "h0": np.zeros((2, B, H), np.float32),
        "theta0": np.zeros((B, J), np.float32),
        "omega0": np.zeros((B, J), np.float32),
    }
    print(kernel(**demo).shape)


# revision 14
# speedup vs baseline: 1.4562x; 1.4562x over previous
"""Trainium2 Bass kernel for nn_ActivationAndBiophysModel.

2-layer GRU (H=512) + FC + antagonist-muscle biophysics, T=512 steps, B=64.

Strategy notes (why this shape):
- The recurrence is strictly sequential over T=512; the per-step compute is
  small-matrix. On-chip collectives have a ~4.6us floor per call, so any
  cross-core exchange per timestep (1024 of them) costs more than the whole
  computation. Every core therefore runs the full model replicated; core 0's
  output is returned.
- Matmuls use "Form A": stationary operand = h^T (cheap to load, [K<=128, 64]),
  moving operand = weight panels streamed at 1 col/cycle. Cost is then just
  (weight elements / 128) cycles per step, independent of batch size <= 128.
- h^T is regenerated each step from the batch-major h via PE transpose.
- All biases are folded into ones-row matmuls (no extra vector ops).
- The biophysics integrator is algebraically collapsed into 9 fused DVE ops
  using scalar_tensor_tensor.
"""

import sys

for p in ("/opt/trn_rl_repo", "/opt/pypackages"):
    if p not in sys.path:
        sys.path.insert(0, p)

import numpy as np  # noqa: E402

B, T, IN, H, J = 64, 512, 16, 512, 8
HG = 3 * H  # 1536 gate rows

# muscle / joint constants (bilinearInit in the model)
K0, K1, L0m, L1m, Mm = 100.0, 2000.0, 0.06, 0.006, 0.05
Ij, Kj, Bj, DT = 0.004, 5.0, 0.3, 1.0 / 60.0

# collapsed integrator coefficients:
# s = a1+a0, d = a1-a0, p = s*d
# om' = ALPHA*om + BETA*d + GAMMA*p + DELTA*th + EPS*(s*th);  th' = th + DT*om'
_c = DT / Ij
ALPHA = 1.0 - _c * Bj
BETA = _c * Mm * (K0 * L1m + K1 * L0m)
GAMMA = _c * Mm * K1 * L1m
DELTA = _c * (-(2.0 * Mm * Mm * K0) - Kj)
EPS = _c * (-(Mm * Mm * K1))


def _build(nc, bass, tile, mybir, T_run, mv_dt, unroll_dma=8):
    """Emit the full unrolled program into nc."""
    f32 = mybir.dt.float32
    AF = mybir.ActivationFunctionType
    Alu = mybir.AluOpType

    def mm(out, lhsT, rhs, **kw):
        nc.tensor.matmul(out, lhsT.bitcast(mv_dt), rhs.bitcast(mv_dt), **kw)

    # ---- DRAM parameters -------------------------------------------------
    xT_d = nc.declare_dram_parameter("xT", [IN + 1, T_run, B], mv_dt, isOutput=False)
    w0x_d = nc.declare_dram_parameter("w0x", [IN + 1, HG], mv_dt, isOutput=False)
    w0h_d = nc.declare_dram_parameter("w0h", [4, 128, HG], mv_dt, isOutput=False)
    w1i_d = nc.declare_dram_parameter("w1i", [4, 128, HG], mv_dt, isOutput=False)
    w1h_d = nc.declare_dram_parameter("w1h", [4, 128, HG], mv_dt, isOutput=False)
    wfc_d = nc.declare_dram_parameter("wfc", [4, 128, 2 * J], mv_dt, isOutput=False)
    brows_d = nc.declare_dram_parameter("brows", [1, 2576 + B], mv_dt, isOutput=False)
    ident_d = nc.declare_dram_parameter("ident", [B, B], f32, isOutput=False)
    hb0_d = nc.declare_dram_parameter("hb0", [2, B, H], f32, isOutput=False)
    hT0_d = nc.declare_dram_parameter("hT0", [2, 4, 128, B], mv_dt, isOutput=False)
    th0_d = nc.declare_dram_parameter("th0", [B, J], f32, isOutput=False)
    om0_d = nc.declare_dram_parameter("om0", [B, J], f32, isOutput=False)
    out_d = nc.declare_dram_parameter("out", [B, T_run * J], f32, isOutput=True)

    # brows layout offsets
    OB1RZ, OB1IN, OB1HN, OB0HN, OBFC = 0, 1024, 1536, 2048, 2560

    with tile.TileContext(nc) as tc:
        with (
            tc.tile_pool(name="wpool", bufs=1) as wp,
            tc.tile_pool(name="xpool", bufs=8) as xp,
            tc.tile_pool(name="state", bufs=2) as sp,
            tc.tile_pool(name="gates", bufs=2) as gp,
            tc.tile_pool(name="bp", bufs=2) as bp,
            tc.tile_pool(name="prz", bufs=2, space="PSUM") as prz,
            tc.tile_pool(name="psm", bufs=4, space="PSUM") as psm,
        ):
            # ---- load constants/weights once -----------------------------
            w0x = wp.tile([IN + 1, HG], mv_dt)
            nc.sync.dma_start(w0x[:], w0x_d[:])
            w0h = wp.tile([128, 4, HG], mv_dt)
            w1i = wp.tile([128, 4, HG], mv_dt)
            w1h = wp.tile([128, 4, HG], mv_dt)
            wfc = wp.tile([128, 4, 2 * J], mv_dt)
            for c in range(4):
                nc.sync.dma_start(w0h[:, c, :], w0h_d[c])
                nc.sync.dma_start(w1i[:, c, :], w1i_d[c])
                nc.sync.dma_start(w1h[:, c, :], w1h_d[c])
                nc.sync.dma_start(wfc[:, c, :], wfc_d[c])
            brows = wp.tile([1, 2576 + B], mv_dt)
            nc.sync.dma_start(brows[:], brows_d[:])
            ident = wp.tile([B, B], f32)
            nc.sync.dma_start(ident[:], ident_d[:])
            out_sb = wp.tile([B, T_run * J], f32)
            ones = brows[:, 2576 : 2576 + B]

            # ---- initial state -------------------------------------------
            h0b = sp.tile([B, H], f32, tag="h0b")
            h1b = sp.tile([B, H], f32, tag="h1b")
            nc.sync.dma_start(h0b[:], hb0_d[0])
            nc.sync.dma_start(h1b[:], hb0_d[1])
            h0T = sp.tile([128, 4 * B], mv_dt, tag="h0T")
            h1T = sp.tile([128, 4 * B], mv_dt, tag="h1T")
            for c in range(4):
                nc.sync.dma_start(h0T[:, c * B : (c + 1) * B], hT0_d[0, c])
                nc.sync.dma_start(h1T[:, c * B : (c + 1) * B], hT0_d[1, c])
            th_init = sp.tile([B, J], f32, tag="th")
            nc.sync.dma_start(th_init[:], th0_d[:])
            om = sp.tile([B, J], f32, tag="om")
            nc.sync.dma_start(om[:], om0_d[:])
            th_ap = th_init[:]

            xts = [None] * (T_run + 2)

            def dma_x(t):
                if t < T_run and xts[t] is None:
                    xt = xp.tile([IN + 1, B], mv_dt, tag="xt")
                    nc.sync.dma_start(xt[:], xT_d[:, t, :])
                    xts[t] = xt

            def l0_rz_mms(rz, h0T_ap, xt):
                sl = [rz[:, 0:512], rz[:, 512:1024]]
                for c in range(4):
                    st = h0T_ap[:, c * B : (c + 1) * B]
                    for ns in range(2):
                        mm(sl[ns], st, w0h[:, c, ns * 512 : ns * 512 + 512],
                           start=(c == 0), stop=False)
                for ns in range(2):
                    mm(sl[ns], xt[:], w0x[:, ns * 512 : ns * 512 + 512],
                       start=False, stop=True)

            def l0_ni_mms(hn, inn, h0T_ap, xt):
                for c in range(4):
                    mm(hn[:], h0T_ap[:, c * B : (c + 1) * B], w0h[:, c, 1024:1536],
                       start=(c == 0), stop=False)
                mm(inn[:], xt[:], w0x[:, 1024:1536], start=True, stop=True)
                mm(hn[:], ones, brows[:, OB0HN : OB0HN + 512],
                   start=False, stop=True)

            def l1_hh_mms(rz, hn, h1T_ap):
                sl = [rz[:, 0:512], rz[:, 512:1024]]
                for c in range(4):
                    st = h1T_ap[:, c * B : (c + 1) * B]
                    for ns in range(2):
                        mm(sl[ns], st, w1h[:, c, ns * 512 : ns * 512 + 512],
                           start=(c == 0), stop=False)
                    mm(hn[:], st, w1h[:, c, 1024:1536], start=(c == 0), stop=False)

            def l1_ih_mms(rz, hn, inn, h0T_ap):
                # rz part + its stop-biases first so sigmoid1 launches early
                sl = [rz[:, 0:512], rz[:, 512:1024]]
                for c in range(4):
                    st = h0T_ap[:, c * B : (c + 1) * B]
                    for ns in range(2):
                        mm(sl[ns], st, w1i[:, c, ns * 512 : ns * 512 + 512],
                           start=False, stop=False)
                for ns in range(2):
                    mm(sl[ns], ones, brows[:, OB1RZ + ns * 512 : OB1RZ + ns * 512 + 512],
                       start=False, stop=True)
                mm(hn[:], ones, brows[:, OB1HN : OB1HN + 512], start=False, stop=True)
                for c in range(4):
                    mm(inn[:], h0T_ap[:, c * B : (c + 1) * B], w1i[:, c, 1024:1536],
                       start=(c == 0), stop=False)
                mm(inn[:], ones, brows[:, OB1IN : OB1IN + 512], start=False, stop=True)

            def gru_vec(rz, hn, inn, lo, hb_prev, hb_tag):
                r_ = gp.tile([B, H], f32, tag=f"r{lo}")
                nc.scalar.activation(r_[:], rz[:, 0:512], AF.Sigmoid)
                z_ = gp.tile([B, H], f32, tag=f"z{lo}")
                nc.scalar.activation(z_[:], rz[:, 512:1024], AF.Sigmoid)
                zb = gp.tile([B, H], f32, tag=f"zb{lo}")
                nc.scalar.activation(zb[:], rz[:, 512:1024], AF.Sigmoid, scale=-1.0)
                q2 = gp.tile([B, H], f32, tag=f"q2{lo}")
                nc.vector.tensor_mul(q2[:], z_[:], hb_prev[:])
                t_ = gp.tile([B, H], f32, tag=f"t{lo}")
                nc.vector.tensor_mul(t_[:], r_[:], hn[:])
                u_ = gp.tile([B, H], f32, tag=f"u{lo}")
                nc.vector.tensor_add(u_[:], t_[:], inn[:])
                n_ = gp.tile([B, H], f32, tag=f"n{lo}")
                nc.scalar.activation(n_[:], u_[:], AF.Tanh)
                q1 = gp.tile([B, H], f32, tag=f"q1{lo}")
                nc.vector.tensor_mul(q1[:], zb[:], n_[:])
                hb = sp.tile([B, H], f32, tag=hb_tag)
                nc.vector.tensor_add(hb[:], q1[:], q2[:])
                return hb

            def transpose_h(hb, hT_tag):
                ph = psm.tile([128, 4 * B], f32, tag="sm")
                for c in range(4):
                    nc.tensor.transpose(ph[:, c * B : (c + 1) * B],
                                        hb[:, c * 128 : (c + 1) * 128], ident[:])
                hT = sp.tile([128, 4 * B], mv_dt, tag=hT_tag)
                nc.scalar.activation(hT[:], ph[:], AF.Copy)
                return hT

            # ---- prologue: step 0 gate matmuls ---------------------------
            dma_x(0)
            dma_x(1)
            rz0 = prz.tile([B, 1024], f32, tag="rz")
            l0_rz_mms(rz0, h0T, xts[0])
            hn0 = psm.tile([B, 512], f32, tag="sm")
            inn0 = psm.tile([B, 512], f32, tag="sm")
            l0_ni_mms(hn0, inn0, h0T, xts[0])
            rz1 = prz.tile([B, 1024], f32, tag="rz")
            hn1 = psm.tile([B, 512], f32, tag="sm")
            l1_hh_mms(rz1, hn1, h1T)

            # ---- time loop (fully unrolled, 2-deep software pipeline) ----
            for t in range(T_run):
                dma_x(t + 2)
                last = t + 1 >= T_run

                # vec layer0(t) + transpose
                h0b = gru_vec(rz0, hn0, inn0, 0, h0b, "h0b")
                h0T = transpose_h(h0b, "h0T")

                # phase A: layer1(t) ih matmuls, then layer0(t+1) gate matmuls
                inn1 = psm.tile([B, 512], f32, tag="sm")
                l1_ih_mms(rz1, hn1, inn1, h0T)
                if not last:
                    rz0_n = prz.tile([B, 1024], f32, tag="rz")
                    l0_rz_mms(rz0_n, h0T, xts[t + 1])
                    hn0 = psm.tile([B, 512], f32, tag="sm")
                    inn0 = psm.tile([B, 512], f32, tag="sm")
                    l0_ni_mms(hn0, inn0, h0T, xts[t + 1])

                # vec layer1(t) + transpose (overlaps phase A on PE)
                h1b = gru_vec(rz1, hn1, inn1, 1, h1b, "h1b")
                h1T = transpose_h(h1b, "h1T")

                # phase B: fc(t); layer1(t+1) hh
                ps_fc = psm.tile([B, 2 * J], f32, tag="sm")
                fc = ps_fc[:, :]
                for c in range(4):
                    mm(fc, h1T[:, c * B : (c + 1) * B], wfc[:, c, :],
                       start=(c == 0), stop=False)
                mm(fc, ones, brows[:, OBFC : OBFC + 2 * J], start=False, stop=True)
                if not last:
                    rz1_n = prz.tile([B, 1024], f32, tag="rz")
                    hn1 = psm.tile([B, 512], f32, tag="sm")
                    l1_hh_mms(rz1_n, hn1, h1T)
                    rz0, rz1 = rz0_n, rz1_n

                # fc sigmoid + biophysics
                a_s = bp.tile([B, J, 2], f32, tag="as")
                nc.scalar.activation(a_s[:], fc, AF.Sigmoid)
                a0 = a_s[:, :, 0]
                a1 = a_s[:, :, 1]
                s_ = bp.tile([B, J], f32, tag="s")
                nc.gpsimd.tensor_add(s_[:], a1, a0)
                dd = bp.tile([B, J], f32, tag="dd")
                nc.gpsimd.tensor_sub(dd[:], a1, a0)
                p_ = bp.tile([B, J], f32, tag="p")
                nc.gpsimd.tensor_mul(p_[:], s_[:], dd[:])
                w_ = bp.tile([B, J], f32, tag="w")
                nc.vector.scalar_tensor_tensor(
                    w_[:], dd[:], BETA / GAMMA, p_[:], Alu.mult, Alu.add)
                v_ = bp.tile([B, J], f32, tag="v")
                nc.vector.tensor_scalar(v_[:], s_[:], EPS, DELTA, Alu.mult, Alu.add)
                u2 = bp.tile([B, J], f32, tag="u2")
                nc.vector.tensor_mul(u2[:], v_[:], th_ap)
                q_ = bp.tile([B, J], f32, tag="q")
                nc.vector.scalar_tensor_tensor(
                    q_[:], w_[:], GAMMA, u2[:], Alu.mult, Alu.add)
                om_new = sp.tile([B, J], f32, tag="om")
                nc.vector.scalar_tensor_tensor(
                    om_new[:], om[:], ALPHA, q_[:], Alu.mult, Alu.add)
                om = om_new
                th_new = out_sb[:, t * J : (t + 1) * J]
                nc.vector.scalar_tensor_tensor(
                    th_new, om[:], DT, th_ap, Alu.mult, Alu.add)
                th_ap = th_new

            nc.sync.dma_start(out_d[:], out_sb[:])
    return nc


_NC_CACHE = {}


def _get_nc(T_run):
    key = T_run
    if key in _NC_CACHE:
        return _NC_CACHE[key]
    from concourse import bass, bacc, tile

    mybir = bass.mybir
    mv_dt = mybir.dt.float32r  # fp32 data, 1 cycle/row streaming for N>=256
    nc = bacc.Bacc(None, target_bir_lowering=False)
    _build(nc, bass, tile, mybir, T_run, mv_dt)
    nc.compile()
    _NC_CACHE[key] = nc
    return nc


def _prep_inputs(x, W_ih0, W_hh0, b_ih0, b_hh0, W_ih1, W_hh1, b_ih1, b_hh1,
                 fc_W, fc_b, h0, theta0, omega0):
    T_run = x.shape[1]
    f = np.float32
    xT = np.concatenate(
        [np.ascontiguousarray(x.transpose(2, 1, 0)),
         np.ones((1, T_run, B), f)], axis=0).astype(f)  # [17, T, 64]
    b0rz = (b_ih0 + b_hh0)[:1024]
    w0x = np.concatenate(
        [W_ih0.T, np.concatenate([b0rz, b_ih0[1024:]])[None, :]], axis=0
    ).astype(f)  # [17, 1536]
    w0h = np.ascontiguousarray(W_hh0.T.reshape(4, 128, HG)).astype(f)
    w1i = np.ascontiguousarray(W_ih1.T.reshape(4, 128, HG)).astype(f)
    w1h = np.ascontiguousarray(W_hh1.T.reshape(4, 128, HG)).astype(f)
    wfc = np.ascontiguousarray(fc_W.T.reshape(4, 128, 2 * J)).astype(f)
    brows = np.zeros((1, 2576 + B), f)
    brows[0, 2576:] = 1.0
    brows[0, 0:1024] = (b_ih1 + b_hh1)[:1024]
    brows[0, 1024:1536] = b_ih1[1024:]
    brows[0, 1536:2048] = b_hh1[1024:]
    brows[0, 2048:2560] = b_hh0[1024:]
    brows[0, 2560:2576] = fc_b
    hT0 = np.ascontiguousarray(
        np.stack([h0[0].T.reshape(4, 128, B), h0[1].T.reshape(4, 128, B)])
    ).astype(f)
    return {
        "xT": xT, "w0x": w0x, "w0h": w0h, "w1i": w1i, "w1h": w1h,
        "wfc": wfc, "brows": brows, "ident": np.eye(B, dtype=f),
        "hb0": h0.astype(f), "hT0": hT0,
        "th0": theta0.astype(f), "om0": omega0.astype(f),
    }


def _install_loud_hook():
    """Surface compile-hook exceptions (XLA otherwise swallows them)."""
    import traceback

    from concourse import bass2jax

    if getattr(bass2jax, "_loud_hook_installed", False):
        return
    orig = bass2jax.neuronx_cc_hook

    def loud(*a, **k):
        try:
            return orig(*a, **k)
        except BaseException:
            traceback.print_exc()
            raise

    bass2jax.neuronx_cc_hook = loud
    bass2jax._loud_hook_installed = True

    # Enable walrus's LDWEIGHTS-dedup pass (concourse hardcodes it off).
    # Our matmuls are chunk-major so consecutive MMs share the stationary
    # operand; the dedup removes ~half the weight-load traffic.
    import os

    if os.environ.get("KERNEL_LDW_OPT", "1") == "1":
        from concourse import bass_utils as _bu

        if not getattr(_bu, "_ldw_patch", False):
            _orig_rc = _bu.run_command

            def _rc(cmd, **kw):
                cmd = [c.replace("--enable-ldw-opt=false", "--enable-ldw-opt=true")
                       if isinstance(c, str) else c for c in cmd]
                return _orig_rc(cmd, **kw)

            _bu.run_command = _rc
            _bu._ldw_patch = True


def run(inputs, **spmd_kwargs):
    from concourse.bass_utils import run_bass_kernel_spmd

    _install_loud_hook()

    inputs = {k: np.asarray(v) for k, v in inputs.items()}
    T_run = inputs["x"].shape[1]
    nc = _get_nc(T_run)
    in_map = _prep_inputs(**inputs)
    res = run_bass_kernel_spmd(nc, [in_map] * 8, core_ids=list(range(8)),
                               **spmd_kwargs)
    out = res.results[0]["out"].reshape(B, T_run, J).astype(np.float32)
    return out, res


def kernel(**inputs):
    return run(inputs)[0]


if __name__ == "__main__":
    rs = np.random.RandomState(0)
    demo = {
        "x": rs.randn(B, 8, IN).astype(np.float32),
        "W_ih0": 0.04 * rs.randn(HG, IN).astype(np.float32),
        "W_hh0": 0.04 * rs.randn(HG, H).astype(np.float32),
        "b_ih0": 0.04 * rs.randn(HG).astype(np.float32),
        "b_hh0": 0.04 * rs.randn(HG).astype(np.float32),
        "W_ih1": 0.04 * rs.randn(HG, H).astype(np.float32),
        "W_hh1": 0.04 * rs.randn(HG, H).astype(np.float32),
        "b_ih1": 0.04 * rs.randn(HG).astype(np.float32),
        "b_hh1": 0.04 * rs.randn(HG).astype(np.float32),
        "fc_W": 0.04 * rs.randn(2 * J, H).astype(np.float32),
        "fc_b": 0.04 * rs.randn(2 * J).astype(np.float32),
        "h0": np.zeros((2, B, H), np.float32),
        "theta0": np.zeros((B, J), np.float32),
        "omega0": np.zeros((B, J), np.float32),
    }
    print(kernel(**demo).shape)
